# revision 36
# baseline (speedup 1.0000x reference)
"""Trainium2 Bass kernel for nn_MhsLayer (biaffine pairwise logits).

Math:
  u = x @ Wu + bu ; v = x @ Wv + bv
  pu = u @ Wuv[:in] ; pv = v @ Wuv[in:]
  logits[b,r,i,j] = pu[b,j,r] + pv[b,i,r], masked to NEG where mask[i]==0 or mask[j]==0

The linear chain folds on host into Af = [Wu@Wuv[:in] | Wv@Wuv[in:]] (256x8) and
cf (8,).  On device, per core (one batch element per core):
  1. x (1024x256) is DMA'd in and PE-transposed to xT (256x1024).
  2. puv^T = Af^T @ x^T + cf (8x1024, f32 matmul with a ones-row bias fold),
     masked by m via an elementwise multiply.
  3. puv^T is split into hi/mid/lo bf16 parts (24-bit mantissa coverage) so the
     bulk broadcast-add can run as a bf16 rank-8 matmul at 1 cycle/row:
       out[i,j] = m_i*pvm_i*m_j + m_i*pum_j + NEG*1 + (1e-12*m_i)*m_j
     which equals m_i*m_j*(pv_i+pu_j) + NEG*(1-m_i*m_j) exactly.
  4. 64 matmuls [128x512] -> PSUM -> DVE/ACT copy -> SBUF -> DMA out (16 MiB),
     output DMAs alternating between the Sync and Scalar HWDGE queues.

A dozen dummy bf16 matmuls run during the input-DMA window to warm the PE HAM
clock gate so the real matmuls run at 2.4 GHz.
"""

import sys

import numpy as np

if "/opt/trn_rl_repo" not in sys.path:
    sys.path.insert(0, "/opt/trn_rl_repo")

import ml_dtypes

B, L, IN, OUT = 8, 1024, 256, 4
NEG = -1e-12
N_CORES = 8
BF16 = ml_dtypes.bfloat16


def build_nc():
    """Build the per-core Bass program (SPMD: same program, per-core inputs)."""
    import concourse.bass as bass
    import concourse.tile as tile
    from concourse import bacc, mybir

    f32 = mybir.dt.float32
    f32r = mybir.dt.float32r
    bf16 = mybir.dt.bfloat16

    nc = bacc.Bacc("TRN2", target_bir_lowering=False, debug=False, num_devices=1)

    x0_d = nc.dram_tensor("x0", (IN // 2, L + 4 * OUT), f32, kind="ExternalInput").ap()
    x1_d = nc.dram_tensor("x1", (IN // 2, L), f32, kind="ExternalInput").ap()
    m8_d = nc.dram_tensor("m8cf", (2 * OUT, L + 1), f32, kind="ExternalInput").ap()
    mb_d = nc.dram_tensor("mb", (1, L), bf16, kind="ExternalInput").ap()
    pn_d = nc.dram_tensor("pn", (1, L), bf16, kind="ExternalInput").ap()
    cb_d = nc.dram_tensor("cb", (2, L), bf16, kind="ExternalInput").ap()
    out_d = nc.dram_tensor("out", (OUT, L, L), f32, kind="ExternalOutput").ap()

    NT = L // 128  # 8 token tiles
    KC = IN // 128  # 2 feature chunks

    with tile.TileContext(nc) as tc:
        with (
            tc.tile_pool(name="const", bufs=1) as const_pool,
            tc.tile_pool(name="xt", bufs=1) as xt_pool,
            tc.tile_pool(name="small", bufs=1) as small_pool,
            tc.tile_pool(name="obuf", bufs=10) as obuf_pool,
        ):
            # operand tensors for the bulk rank-6 matmul, assembled below.
            # LHS_CAT [6, 4*L]: block r: p0 pvm_hi, p1 pvm_mid, p2..3 m,
            #                   p4 ones, p5 1e-12*m
            # RHS_CAT [6, 4*L]: block r: p0..1 m, p2 pum_hi, p3 pum_mid,
            #                   p4 -1e-12, p5 m
            lhs_cat = small_pool.tile([6, OUT * L], bf16, tag="lhs_cat")
            rhs_cat = small_pool.tile([6, OUT * L], bf16, tag="rhs_cat")

            # ---- PE warmup: keep the HAM clock gate open while inputs DMA in
            with tc.tile_pool(name="warm", bufs=1, space="PSUM") as warm_pool:
                wtile = const_pool.tile([128, 256], bf16, tag="wtile")
                nc.vector.memset(wtile[:], 0.0)
                wp = warm_pool.tile([128, 256], f32, tag="wp")
                for _ in range(14):
                    nc.tensor.matmul(wp[:], wtile[:, :128], wtile[:], start=True, stop=True)

            # ---- input DMAs: xt0 carries the folded weights as 16 extra
            # columns (one clean 4KB+64B-per-row DMA); m8cf carries the mask
            # broadcast rows plus the bias column
            x0t = xt_pool.tile([128, L + 4 * OUT], f32, tag="x0t")
            nc.sync.dma_start(x0t[0:64, :], x0_d[0:64, :])
            nc.scalar.dma_start(x0t[64:128, :], x0_d[64:128, :])
            x1t = xt_pool.tile([128, L], f32, tag="x1t")
            nc.sync.dma_start(x1t[0:64, :], x1_d[0:64, :])
            nc.scalar.dma_start(x1t[64:128, :], x1_d[64:128, :])
            m8t = const_pool.tile([2 * OUT, L + 1], f32, tag="m8t")
            nc.sync.dma_start(m8t[:], m8_d)
            xt = [x0t, x1t]
            af_sb = x0t[:, L : L + 4 * OUT]
            m8 = m8t[:, 0:L]
            cf_sb = m8t[:, L : L + 1]

            # mask/const rows have no compute deps: DMA them first (gpsimd SWDGE)
            nc.gpsimd.dma_start(lhs_cat[2:4, :], mb_d.partition_broadcast(2 * OUT))
            nc.gpsimd.dma_start(rhs_cat[0:2, :], mb_d.partition_broadcast(2 * OUT))
            nc.gpsimd.dma_start(rhs_cat[5:6, :], mb_d.partition_broadcast(OUT))
            nc.gpsimd.dma_start(lhs_cat[5:6, :], pn_d.partition_broadcast(OUT))
            nc.gpsimd.dma_start(lhs_cat[4:5, :], cb_d[0:1, :].partition_broadcast(OUT))
            nc.gpsimd.dma_start(rhs_cat[4:5, :], cb_d[1:2, :].partition_broadcast(OUT))


            puvm = small_pool.tile([2 * OUT, L], f32, tag="puvm")
            hi = small_pool.tile([2 * OUT, L], bf16, tag="hi")
            mid = small_pool.tile([2 * OUT, L], bf16, tag="mid")

            with tc.tile_pool(name="ppsum", bufs=2, space="PSUM") as ppsum_pool:
                lhs_v = lhs_cat[:].rearrange("p (r t) -> p r t", r=OUT)
                rhs_v = rhs_cat[:].rearrange("p (r t) -> p r t", r=OUT)

                def half_chain(jh):
                    # projection + mask+bias + 2-way bf16 split + gathers
                    pp = ppsum_pool.tile([2 * OUT, 512], f32, tag="pp")
                    sl = slice(jh * 512, (jh + 1) * 512)
                    nc.tensor.matmul(
                        pp[:], af_sb[:, 0 : 2 * OUT], xt[0][:, sl], start=True, stop=False
                    )
                    nc.tensor.matmul(
                        pp[:],
                        af_sb[:, 2 * OUT : 4 * OUT],
                        xt[1][:, sl],
                        start=False,
                        stop=True,
                    )
                    nc.vector.scalar_tensor_tensor(
                        puvm[:, sl],
                        pp[:],
                        cf_sb,
                        m8[:, sl],
                        mybir.AluOpType.add,
                        mybir.AluOpType.mult,
                    )
                    nc.vector.tensor_copy(hi[:, sl], puvm[:, sl])
                    nc.vector.tensor_sub(mid[:, sl], puvm[:, sl], hi[:, sl])
                    gather_engs = (nc.sync, nc.gpsimd, nc.scalar, nc.gpsimd)
                    ge = iter(gather_engs)
                    for t, dst_p in ((hi, 0), (mid, 1)):
                        next(ge).dma_start(
                            lhs_v[dst_p : dst_p + 1, :, sl], t[OUT : 2 * OUT, sl]
                        )
                        next(ge).dma_start(
                            rhs_v[dst_p + 2 : dst_p + 3, :, sl], t[0:OUT, sl]
                        )

                half_chain(0)
                half_chain(1)

            # ---- bulk: out[i,j] tiles; half-0-only tiles first ----
            with tc.tile_pool(name="bpsum", bufs=6, space="PSUM") as bpsum_pool:
                obufs = {}
                k = 0

                def bulk_half(n, r, jh):
                    nonlocal k
                    if (n, r) not in obufs:
                        obufs[(n, r)] = obuf_pool.tile(
                            [128, L], f32, tag="ob", name=f"ob_{n}_{r}"
                        )
                    ob = obufs[(n, r)]
                    bp = bpsum_pool.tile([128, 512], f32, tag="bp", name=f"bp_{n}_{r}_{jh}")
                    nc.tensor.matmul(
                        bp[:],
                        lhs_cat[:, r * L + n * 128 : r * L + (n + 1) * 128],
                        rhs_cat[:, r * L + jh * 512 : r * L + (jh + 1) * 512],
                        start=True,
                        stop=True,
                    )
                    sl = slice(jh * 512, (jh + 1) * 512)
                    if jh == 0:
                        nc.scalar.copy(ob[:, sl], bp[:])
                    else:
                        nc.vector.tensor_copy(ob[:, sl], bp[:])

                def flush(n, r):
                    nonlocal k
                    ob = obufs.pop((n, r))
                    dst = out_d[r, n * 128 : (n + 1) * 128, :]
                    if k % 2 == 0:
                        nc.sync.dma_start(dst, ob[:])
                    else:
                        nc.scalar.dma_start(dst, ob[:])
                    k += 1

                for n in range(NT):
                    for r in range(OUT):
                        bulk_half(n, r, 0)
                        bulk_half(n, r, 1)
                        flush(n, r)

    nc.compile()
    return nc


_NC = None


def _get_nc():
    global _NC
    if _NC is None:
        _NC = build_nc()
    return _NC


def make_in_maps(inputs, mask, Wu, bu, Wv, bv, Wuv):
    Af = np.concatenate(
        [
            Wu.astype(np.float64) @ Wuv[:IN].astype(np.float64),
            Wv.astype(np.float64) @ Wuv[IN:].astype(np.float64),
        ],
        axis=1,
    ).astype(np.float32)  # (256, 8)
    # two k-chunks side by side: [128, 16]
    Af2 = np.concatenate([Af[:128], Af[128:]], axis=1)
    cf = np.concatenate(
        [
            bu.astype(np.float64) @ Wuv[:IN].astype(np.float64),
            bv.astype(np.float64) @ Wuv[IN:].astype(np.float64),
        ]
    ).astype(np.float32).reshape(2 * OUT, 1)
    cb = np.stack([np.ones(L, dtype=BF16), np.full(L, np.float32(NEG), dtype=BF16)])
    in_maps = []
    for b in range(B):
        mf = mask[b].astype(np.float32).reshape(1, L)
        mb = mf.astype(BF16)
        pn = (mf * np.float32(1e-12)).astype(BF16)
        xT = inputs[b].T
        x0 = np.concatenate([xT[:128], Af2], axis=1)
        m8cf = np.concatenate(
            [np.broadcast_to(mf, (2 * OUT, L)), np.broadcast_to(cf, (2 * OUT, 1))],
            axis=1,
        )
        in_maps.append(
            {
                "x0": np.ascontiguousarray(x0),
                "x1": np.ascontiguousarray(xT[128:]),
                "m8cf": np.ascontiguousarray(m8cf),
                "mb": mb,
                "pn": pn,
                "cb": cb,
            }
        )
    return in_maps


def kernel(inputs, mask, Wu, bu, Wv, bv, Wuv):
    from concourse import bass_utils

    inputs = np.asarray(inputs)
    mask = np.asarray(mask)
    nc = _get_nc()
    in_maps = make_in_maps(inputs, mask, Wu, bu, Wv, bv, Wuv)
    res = bass_utils.run_bass_kernel_spmd(nc, in_maps, core_ids=list(range(N_CORES)))
    out = np.stack([res.results[c]["out"] for c in range(N_CORES)], axis=0)
    return out


# revision 37
# speedup vs baseline: 1.0612x; 1.0612x over previous
"""Trainium2 Bass kernel for nn_MhsLayer (biaffine pairwise logits).

Math:
  u = x @ Wu + bu ; v = x @ Wv + bv
  pu = u @ Wuv[:in] ; pv = v @ Wuv[in:]
  logits[b,r,i,j] = pu[b,j,r] + pv[b,i,r], masked to NEG where mask[i]==0 or mask[j]==0

The linear chain folds on host into Af = [Wu@Wuv[:in] | Wv@Wuv[in:]] (256x8) and
cf (8,).  On device, per core (one batch element per core):
  1. x (1024x256) is DMA'd in and PE-transposed to xT (256x1024).
  2. puv^T = Af^T @ x^T + cf (8x1024, f32 matmul with a ones-row bias fold),
     masked by m via an elementwise multiply.
  3. puv^T is split into hi/mid/lo bf16 parts (24-bit mantissa coverage) so the
     bulk broadcast-add can run as a bf16 rank-8 matmul at 1 cycle/row:
       out[i,j] = m_i*pvm_i*m_j + m_i*pum_j + NEG*1 + (1e-12*m_i)*m_j
     which equals m_i*m_j*(pv_i+pu_j) + NEG*(1-m_i*m_j) exactly.
  4. 64 matmuls [128x512] -> PSUM -> DVE/ACT copy -> SBUF -> DMA out (16 MiB),
     output DMAs alternating between the Sync and Scalar HWDGE queues.

A dozen dummy bf16 matmuls run during the input-DMA window to warm the PE HAM
clock gate so the real matmuls run at 2.4 GHz.
"""

import sys

import numpy as np

if "/opt/trn_rl_repo" not in sys.path:
    sys.path.insert(0, "/opt/trn_rl_repo")

import ml_dtypes

B, L, IN, OUT = 8, 1024, 256, 4
NEG = -1e-12
N_CORES = 8
BF16 = ml_dtypes.bfloat16


def build_nc():
    """Build the per-core Bass program (SPMD: same program, per-core inputs)."""
    import concourse.bass as bass
    import concourse.tile as tile
    from concourse import bacc, mybir

    f32 = mybir.dt.float32
    f32r = mybir.dt.float32r
    bf16 = mybir.dt.bfloat16

    nc = bacc.Bacc("TRN2", target_bir_lowering=False, debug=False, num_devices=1)

    x0_d = nc.dram_tensor("x0", (IN // 2, L + 4 * OUT), f32, kind="ExternalInput").ap()
    x1_d = nc.dram_tensor("x1", (IN // 2, L), f32, kind="ExternalInput").ap()
    m8_d = nc.dram_tensor("m8cf", (2 * OUT, L + 1), f32, kind="ExternalInput").ap()
    mb_d = nc.dram_tensor("mb", (1, L), bf16, kind="ExternalInput").ap()
    pn_d = nc.dram_tensor("pn", (1, L), bf16, kind="ExternalInput").ap()
    cb_d = nc.dram_tensor("cb", (2, L), bf16, kind="ExternalInput").ap()
    out_d = nc.dram_tensor("out", (OUT, L, L), f32, kind="ExternalOutput").ap()

    NT = L // 128  # 8 token tiles
    KC = IN // 128  # 2 feature chunks

    with tile.TileContext(nc) as tc:
        with (
            tc.tile_pool(name="const", bufs=1) as const_pool,
            tc.tile_pool(name="xt", bufs=1) as xt_pool,
            tc.tile_pool(name="small", bufs=1) as small_pool,
            tc.tile_pool(name="obuf", bufs=10) as obuf_pool,
        ):
            # operand tensors for the bulk rank-6 matmul, assembled below.
            # LHS_CAT [6, 4*L]: block r: p0 pvm_hi, p1 pvm_mid, p2..3 m,
            #                   p4 ones, p5 1e-12*m
            # RHS_CAT [6, 4*L]: block r: p0..1 m, p2 pum_hi, p3 pum_mid,
            #                   p4 -1e-12, p5 m
            lhs_cat = small_pool.tile([6, OUT * L], bf16, tag="lhs_cat")
            rhs_cat = small_pool.tile([6, OUT * L], bf16, tag="rhs_cat")

            # ---- PE warmup: keep the HAM clock gate open while inputs DMA in
            with tc.tile_pool(name="warm", bufs=1, space="PSUM") as warm_pool:
                wtile = const_pool.tile([128, 256], bf16, tag="wtile")
                nc.vector.memset(wtile[:], 0.0)
                wp = warm_pool.tile([128, 256], f32, tag="wp")
                for _ in range(14):
                    nc.tensor.matmul(wp[:], wtile[:, :128], wtile[:], start=True, stop=True)

            # ---- input DMAs: xt0 carries the folded weights as 16 extra
            # columns (one clean 4KB+64B-per-row DMA); m8cf carries the mask
            # broadcast rows plus the bias column
            x0t = xt_pool.tile([128, L + 4 * OUT], f32, tag="x0t")
            nc.sync.dma_start(x0t[:], x0_d)
            x1t = xt_pool.tile([128, L], f32, tag="x1t")
            nc.scalar.dma_start(x1t[:], x1_d)
            m8t = const_pool.tile([2 * OUT, L + 1], f32, tag="m8t")
            nc.sync.dma_start(m8t[:], m8_d)
            xt = [x0t, x1t]
            af_sb = x0t[:, L : L + 4 * OUT]
            m8 = m8t[:, 0:L]
            cf_sb = m8t[:, L : L + 1]

            # mask/const rows have no compute deps: DMA them first (gpsimd SWDGE)
            nc.gpsimd.dma_start(lhs_cat[2:4, :], mb_d.partition_broadcast(2 * OUT))
            nc.gpsimd.dma_start(rhs_cat[0:2, :], mb_d.partition_broadcast(2 * OUT))
            nc.gpsimd.dma_start(rhs_cat[5:6, :], mb_d.partition_broadcast(OUT))
            nc.gpsimd.dma_start(lhs_cat[5:6, :], pn_d.partition_broadcast(OUT))
            nc.gpsimd.dma_start(lhs_cat[4:5, :], cb_d[0:1, :].partition_broadcast(OUT))
            nc.gpsimd.dma_start(rhs_cat[4:5, :], cb_d[1:2, :].partition_broadcast(OUT))


            puvm = small_pool.tile([2 * OUT, L], f32, tag="puvm")
            hi = small_pool.tile([2 * OUT, L], bf16, tag="hi")
            mid = small_pool.tile([2 * OUT, L], bf16, tag="mid")

            with tc.tile_pool(name="ppsum", bufs=2, space="PSUM") as ppsum_pool:
                lhs_v = lhs_cat[:].rearrange("p (r t) -> p r t", r=OUT)
                rhs_v = rhs_cat[:].rearrange("p (r t) -> p r t", r=OUT)

                def half_chain(jh):
                    # projection + mask+bias + 2-way bf16 split + gathers
                    pp = ppsum_pool.tile([2 * OUT, 512], f32, tag="pp")
                    sl = slice(jh * 512, (jh + 1) * 512)
                    nc.tensor.matmul(
                        pp[:], af_sb[:, 0 : 2 * OUT], xt[0][:, sl], start=True, stop=False
                    )
                    nc.tensor.matmul(
                        pp[:],
                        af_sb[:, 2 * OUT : 4 * OUT],
                        xt[1][:, sl],
                        start=False,
                        stop=True,
                    )
                    nc.vector.scalar_tensor_tensor(
                        puvm[:, sl],
                        pp[:],
                        cf_sb,
                        m8[:, sl],
                        mybir.AluOpType.add,
                        mybir.AluOpType.mult,
                    )
                    nc.vector.tensor_copy(hi[:, sl], puvm[:, sl])
                    nc.vector.tensor_sub(mid[:, sl], puvm[:, sl], hi[:, sl])
                    gather_engs = (nc.sync, nc.gpsimd, nc.scalar, nc.gpsimd)
                    ge = iter(gather_engs)
                    for t, dst_p in ((hi, 0), (mid, 1)):
                        next(ge).dma_start(
                            lhs_v[dst_p : dst_p + 1, :, sl], t[OUT : 2 * OUT, sl]
                        )
                        next(ge).dma_start(
                            rhs_v[dst_p + 2 : dst_p + 3, :, sl], t[0:OUT, sl]
                        )

                half_chain(0)
                half_chain(1)

            # ---- bulk: out[i,j] tiles; half-0-only tiles first ----
            with tc.tile_pool(name="bpsum", bufs=6, space="PSUM") as bpsum_pool:
                obufs = {}
                k = 0

                def bulk_half(n, r, jh):
                    nonlocal k
                    if (n, r) not in obufs:
                        obufs[(n, r)] = obuf_pool.tile(
                            [128, L], f32, tag="ob", name=f"ob_{n}_{r}"
                        )
                    ob = obufs[(n, r)]
                    bp = bpsum_pool.tile([128, 512], f32, tag="bp", name=f"bp_{n}_{r}_{jh}")
                    nc.tensor.matmul(
                        bp[:],
                        lhs_cat[:, r * L + n * 128 : r * L + (n + 1) * 128],
                        rhs_cat[:, r * L + jh * 512 : r * L + (jh + 1) * 512],
                        start=True,
                        stop=True,
                    )
                    sl = slice(jh * 512, (jh + 1) * 512)
                    if jh == 0:
                        nc.scalar.copy(ob[:, sl], bp[:])
                    else:
                        nc.vector.tensor_copy(ob[:, sl], bp[:])

                def flush(n, r):
                    nonlocal k
                    ob = obufs.pop((n, r))
                    dst = out_d[r, n * 128 : (n + 1) * 128, :]
                    if k % 2 == 0:
                        nc.sync.dma_start(dst, ob[:])
                    else:
                        nc.scalar.dma_start(dst, ob[:])
                    k += 1

                for n in range(NT):
                    for r in range(OUT):
                        bulk_half(n, r, 0)
                        bulk_half(n, r, 1)
                        flush(n, r)

    nc.compile()
    return nc


_NC = None


def _get_nc():
    global _NC
    if _NC is None:
        _NC = build_nc()
    return _NC


def make_in_maps(inputs, mask, Wu, bu, Wv, bv, Wuv):
    Af = np.concatenate(
        [
            Wu.astype(np.float64) @ Wuv[:IN].astype(np.float64),
            Wv.astype(np.float64) @ Wuv[IN:].astype(np.float64),
        ],
        axis=1,
    ).astype(np.float32)  # (256, 8)
    # two k-chunks side by side: [128, 16]
    Af2 = np.concatenate([Af[:128], Af[128:]], axis=1)
    cf = np.concatenate(
        [
            bu.astype(np.float64) @ Wuv[:IN].astype(np.float64),
            bv.astype(np.float64) @ Wuv[IN:].astype(np.float64),
        ]
    ).astype(np.float32).reshape(2 * OUT, 1)
    cb = np.stack([np.ones(L, dtype=BF16), np.full(L, np.float32(NEG), dtype=BF16)])
    in_maps = []
    for b in range(B):
        mf = mask[b].astype(np.float32).reshape(1, L)
        mb = mf.astype(BF16)
        pn = (mf * np.float32(1e-12)).astype(BF16)
        xT = inputs[b].T
        x0 = np.concatenate([xT[:128], Af2], axis=1)
        m8cf = np.concatenate(
            [np.broadcast_to(mf, (2 * OUT, L)), np.broadcast_to(cf, (2 * OUT, 1))],
            axis=1,
        )
        in_maps.append(
            {
                "x0": np.ascontiguousarray(x0),
                "x1": np.ascontiguousarray(xT[128:]),
                "m8cf": np.ascontiguousarray(m8cf),
                "mb": mb,
                "pn": pn,
                "cb": cb,
            }
        )
    return in_maps


def kernel(inputs, mask, Wu, bu, Wv, bv, Wuv):
    from concourse import bass_utils

    inputs = np.asarray(inputs)
    mask = np.asarray(mask)
    nc = _get_nc()
    in_maps = make_in_maps(inputs, mask, Wu, bu, Wv, bv, Wuv)
    res = bass_utils.run_bass_kernel_spmd(nc, in_maps, core_ids=list(range(N_CORES)))
    out = np.stack([res.results[c]["out"] for c in range(N_CORES)], axis=0)
    return out
